# revision 49
# baseline (speedup 1.0000x reference)
"""Trainium2 Bass kernel for nn_CrossAttentionBlock.

Reference computation (B=16384, C=1024, D=128):
    g_x     = x0 @ g_w.T + g_b          # [B, D]
    theta_x = x1 @ theta_w.T + theta_b  # [B, D]
    phi_x   = x1 @ phi_w.T + phi_b      # [B, D]
    f[b,i,j] = phi_x[b,i] * theta_x[b,j]
    attn = softmax(f, axis=-1)
    y[b,i] = sum_j attn[b,i,j] * g_x[b,j]
    out = y @ W_w.T + W_b + x0          # [B, C]

Unnormalized form used on-chip (no max-subtraction needed: |f| <= ~40, exp
fits fp32/bf16 comfortably):
    E_T[j,i] = exp(theta[b,j] * phi[b,i])        (per b, j on partitions)
    num[i] = sum_j g[b,j] * E_T[j,i]   den[i] = sum_j E_T[j,i]
    y[b,i] = num[i] / den[i]

Sharding: pure data parallel over batch across 8 cores (2048 rows/core).

Per-core pipeline:
  P1: theta/phi projections -> [b,d] fp16 tiles; g projection -> g_xT [d,b]
      interleaved with ones into g1 [d, 2b] bf16.
  P2: per-quad rank-1 outer-product matmuls run 3-concurrent via row tiling
      (tile_position strips at partitions 0/32/64, K=4 per strip): each
      12-row f-tile [128, 1536] fp32 (3 PSUM banks) is produced by 3
      concurrent N=512 matmuls, ~3x faster than serial streaming.
      exp splits each f-tile between the Scalar engine (exact EXP, cols
      [0:D1]) and the Vector engine (int16 Schraudolph bit-trick, cols
      [D1:]) so both engines run concurrently on every tile.
      Per-b reduce matmuls (lhsT=E_T_b, rhs=[g|1]) accumulate num/den in
      PSUM; DVE reciprocal+mul produce y_T [d,b] bf16.
  P3: final matmul (lhsT=y_T group, rhs=W_w.T, N=1024) + residual add + DMA.
"""

import os
from contextlib import ExitStack, nullcontext

import numpy as np

import concourse.bass as bass
import concourse.tile as tile
from concourse import bacc
from concourse import mybir

F32 = mybir.dt.float32
F16 = mybir.dt.float16
BF16 = mybir.dt.bfloat16
I16 = mybir.dt.int16

# bf16 Schraudolph exp: bf16_bits(e^f) ~ int16(f * 128*log2(e) + 16250.4).
# ~6% max relative error on weights; softmax ratio cancels most of it.
SCH_A = 128.0 * 1.4426950408889634
SCH_B = 16250.4

NCORES = 8
B, C, D = 16384, 1024, 128
KC = C // 128  # 8 contraction chunks for the projections

NSTRIP = 3          # concurrent row-tiled strips (partitions 0/32/64)
FTILE = 4 * NSTRIP  # batch rows per f/E tile = 12
SBATCH = 384        # rows per realign super-batch (3*128: G-aligned)
NSLOT = SBATCH // FTILE  # 32 outer slots per super-batch
# exp engine interleave: of every 5 f-tiles, ACT (exact EXP) takes 3 and
# DVE (int16 Schraudolph bit-trick) takes 2 — tile-granular so the two
# engines run concurrently (sharing one tile's reads would serialize).
DVE_TILES = (1, 3)


def build_bass(bc: int, reps: int = 1):
    """Build the per-core bass program for a batch slice of `bc` rows."""
    ng = bc // 128  # groups of 128 rows
    qsz = min(bc, 512)
    nq = max(1, bc // qsz)
    nbatch = (bc + SBATCH - 1) // SBATCH
    # tiles per batch: 8 per 96-row chunk (see tile_rows below)
    last_rows = bc - SBATCH * (nbatch - 1)
    n_ftiles = (nbatch - 1) * NSLOT + 8 * ((last_rows + 95) // 96)

    nc = bacc.Bacc(trn_type="TRN2")

    # inputs are pre-swizzled on the host so every DMA lands per-partition
    # contiguous ([p, k*b] rows): x1g[G*128+p, k*128+b] = x1[G*128+b, k*128+p]
    x1g = nc.dram_tensor("x1g", [ng * 128, KC * 128], F16, kind="ExternalInput")
    x0g = nc.dram_tensor("x0g", [nq * 128, KC * qsz], F16, kind="ExternalInput")
    x0r = nc.dram_tensor("x0r", [bc, C], F16, kind="ExternalInput")
    wc = nc.dram_tensor("wc", [128, KC * 2 * D], F16, kind="ExternalInput")
    gwt = nc.dram_tensor("gwt", [128, KC * D], F16, kind="ExternalInput")
    wwt = nc.dram_tensor("wwt", [D, C], BF16, kind="ExternalInput")
    btp = nc.dram_tensor("btp", [128, 2 * D], F32, kind="ExternalInput")
    gb = nc.dram_tensor("gb", [D, 1], F32, kind="ExternalInput")
    zz = nc.dram_tensor("zz", [32, NSLOT * 4 * D], F16, kind="ExternalInput")
    out = nc.dram_tensor("out", [bc, C], F32, kind="ExternalOutput")

    with tile.TileContext(nc) as tc, ExitStack() as ctx:
        singles = ctx.enter_context(tc.tile_pool(name="singles", bufs=1))

        # ---- static weights / constants in SBUF ----
        wc_sb = singles.tile([128, KC, 2 * D], F16)  # [c-part, chunk, 256]
        wc_src = wc[:, :].rearrange("p (k d) -> p k d", k=KC)
        for s0 in range(0, 128, 32):
            nc.sync.dma_start(wc_sb[s0 : s0 + 32], wc_src[s0 : s0 + 32])
        btp_sb = singles.tile([128, 2 * D], F32)
        nc.sync.dma_start(btp_sb, btp[:, :])
        gwt_sb = singles.tile([128, KC, D], F16)
        gwt_src = gwt[:, :].rearrange("p (k d) -> p k d", k=KC)
        for s0 in range(0, 128, 64):
            nc.sync.dma_start(gwt_sb[s0 : s0 + 64], gwt_src[s0 : s0 + 64])
        wwt_sb = singles.tile([128, C], BF16)
        nc.sync.dma_start(wwt_sb, wwt[:, :])
        gb_sb = singles.tile([128, 1], F32)
        nc.sync.dma_start(gb_sb, gb[:, :])

        # persistent per-core activations
        tp16 = singles.tile([128, ng, 2 * D], F16)  # [theta|phi] fp16, [b-part, G, 256]
        g1 = singles.tile([128, 2 * bc], BF16)  # g_xT interleaved with ones [d, 2b]
        y16 = singles.tile([128, bc], BF16)  # y_T [d, b] bf16

        # ping-pong realign buffers for the outer products: strip s uses
        # partitions 32s..32s+4; slot u along the free dim. phbuf holds phi
        # block-diagonally ([32s+p, u*512 + p*128 + c]); off-block stays 0.
        thbuf = [singles.tile([128, NSLOT * D], F16, name=f"thbuf{i}") for i in range(2)]
        phbuf = [
            singles.tile([128, NSLOT * 4 * D], F16, name=f"phbuf{i}") for i in range(2)
        ]
        # The outer matmuls use K=32 per strip with rows 4..31 zero-padded:
        # the zeros contribute nothing (and cost no extra stream cycles —
        # LDWEIGHTS time is column-count), but they keep 96/128 PE array
        # rows active so the HAM activity monitor holds the 2.4 GHz clock
        # (K=4 matmuls read as ~10% activity -> permanent half-clock).
        # Zero-fill via DMAs from a host zeros tensor; realign rewrites only
        # rows 32s..32s+4 each batch, the padding stays zero. phbuf[1]/
        # thbuf[1] zeroing is deferred (first needed ~16 tiles in).
        # Warm-up matmuls use a separate garbage tile (no dependencies).
        for s in range(NSTRIP):
            nc.sync.dma_start(phbuf[0][32 * s : 32 * s + 32, :], zz[:, :])
            nc.sync.dma_start(
                thbuf[0][32 * s + 4 : 32 * s + 32, :], zz[0:28, : NSLOT * D]
            )
        wbuf = singles.tile([128, 5 * D], F16, name="warmbuf")
        nc.gpsimd.memset(wbuf, 0.0)

        rep_ctx = tc.For_i(0, reps, 1) if reps > 1 else nullcontext()
        with rep_ctx:
            with (
                tc.tile_pool(name="xin", bufs=6) as xin,
                tc.tile_pool(name="xg", bufs=2) as xg,
                tc.tile_pool(name="projpsum", bufs=1, space="PSUM") as projpsum,
                tc.tile_pool(name="fpsum", bufs=2, space="PSUM") as fpsum,
                tc.tile_pool(name="ndpsum", bufs=1, space="PSUM") as ndpsum,
                tc.tile_pool(name="epool", bufs=5) as epool,
                tc.tile_pool(name="ndsb", bufs=2) as ndsb,
                tc.tile_pool(name="rec", bufs=2) as rec,
                tc.tile_pool(name="resid", bufs=4) as resid,
                tc.tile_pool(name="osb", bufs=3) as osb,
            ):
                g1v = g1.rearrange("p (b two) -> p b two", two=2)

                # one persistent PSUM bank each, halves ping-ponged per group
                nd_all = ndpsum.tile([128, 512], F32, tag="nd", name="ndall")
                proj_all = projpsum.tile([128, 512], F32, tag="pp", name="ppall")

                x1_tiles = [None] * ng
                x0_tiles = [None] * nq

                def emit_x1_dma(G, nsplit=1):
                    # nsplit>1 issues partition-sliced sub-DMAs that land on
                    # separate queues, cutting arrival latency for the ramp
                    x1_tiles[G] = xin.tile([128, KC, 128], F16, tag="xin", name="xint")
                    src = x1g[G * 128 : (G + 1) * 128, :].rearrange(
                        "p (k b) -> p k b", k=KC
                    )
                    step = 128 // nsplit
                    for s0 in range(0, 128, step):
                        nc.sync.dma_start(
                            x1_tiles[G][s0 : s0 + step], src[s0 : s0 + step]
                        )

                def emit_x0_dma(q, nsplit=1):
                    x0_tiles[q] = xg.tile([128, KC, qsz], F16, tag="xg", name="xgt")
                    src = x0g[q * 128 : (q + 1) * 128, :].rearrange(
                        "p (k b) -> p k b", k=KC
                    )
                    step = 128 // nsplit
                    for s0 in range(0, 128, step):
                        nc.sync.dma_start(
                            x0_tiles[q][s0 : s0 + step], src[s0 : s0 + step]
                        )

                def emit_proj_tp(G):
                    # theta/phi projection for one 128-row group; the PSUM
                    # bank halves ping-pong across groups
                    pt = proj_all[:, (G % 2) * 256 : (G % 2) * 256 + 256]
                    xt = x1_tiles[G]
                    for k in range(KC):
                        nc.tensor.matmul(
                            pt, lhsT=xt[:, k, :], rhs=wc_sb[:, k, :],
                            start=(k == 0), stop=(k == KC - 1),
                        )
                    nc.vector.tensor_add(tp16[:, G, :], pt, btp_sb)

                def emit_proj_g(q):
                    # borrow an f-pool PSUM slot for the g projection
                    gp = fpsum.tile([128, FTILE * 128], F32, tag="f", name="gpt")
                    gp = gp[:, :qsz]
                    xt = x0_tiles[q]
                    for k in range(KC):
                        nc.tensor.matmul(
                            gp, lhsT=gwt_sb[:, k, :], rhs=xt[:, k, :],
                            start=(k == 0), stop=(k == KC - 1),
                        )
                    nc.vector.tensor_scalar_add(
                        g1v[:, q * qsz : (q + 1) * qsz, 0], gp, gb_sb
                    )

                f_tiles = [None] * n_ftiles
                e_tiles = [None] * n_ftiles
                nd_tiles = [None] * ng
                xr_tiles = [None] * ng

                gpq = max(1, qsz // 128)  # groups per g-projection block

                # tile T = (batch Bb, slot u = 8c + v): e-tile col-block
                # j = 4s+p holds row 384*Bb + 96*c + 32*s + 8*p + v. Quads
                # take rows at stride 8 within a 32-row block so the theta
                # realign for a whole (strip, chunk) is ONE natural-order DMA
                # (src = 32 contiguous partitions -> dst [4 parts, 8 slots]).
                def tile_rows(T):
                    Bb, u = T // NSLOT, T % NSLOT
                    c, v = u // 8, u % 8
                    rows = []
                    for s in range(NSTRIP):
                        base = SBATCH * Bb + 96 * c + 32 * s
                        for p in range(4):
                            r = base + 8 * p + v
                            rows.append(r if base < bc and r < bc else -1)
                    return rows

                # per-group remaining-row counts -> completion tile
                g_done_tile = {}
                cnt = [min(bc, (G + 1) * 128) - G * 128 for G in range(ng)]
                for T in range(n_ftiles):
                    for r in tile_rows(T):
                        if r >= 0:
                            cnt[r // 128] -= 1
                            if cnt[r // 128] == 0:
                                g_done_tile.setdefault(r // 128, T)

                def emit_realign(Bb, c, gps_only=False):
                    # chunk (Bb, c): strip s rows [384B+96c+32s, +32).
                    # theta: ONE natural-order DMA per strip (32 contiguous
                    # src partitions -> [4 parts, 8 slots, 128]).
                    # phi: 4 DMAs per strip (8 contiguous src partitions ->
                    # one dst partition, block-diag column offset p*128).
                    # gps_only: startup chunks avoid the sync queue, which is
                    # backed up with input loads there (a >3.4us PE stall
                    # re-throttles the HAM clock for the rest of the run).
                    bi = Bb % 2
                    phv = phbuf[bi][:, :].rearrange("o (t f) -> o t f", f=4 * D)
                    thv = thbuf[bi][:, :].rearrange("o (t f) -> o t f", f=D)
                    for s in range(NSTRIP):
                        base = SBATCH * Bb + 96 * c + 32 * s
                        if base >= bc:
                            return
                        G = base // 128
                        o = base % 128
                        nk = min(32, bc - base)
                        eng = (
                            nc.gpsimd
                            if gps_only or (s + c) % 2 == 0
                            else nc.sync
                        )
                        eng.dma_start(
                            thv[32 * s : 32 * s + (nk + 7) // 8, 8 * c : 8 * c + min(8, nk)]
                            if nk < 32
                            else thv[32 * s : 32 * s + 4, 8 * c : 8 * c + 8],
                            tp16[o : o + nk, G, 0:D],
                        )
                        for p in range(4):
                            if 8 * p >= nk:
                                break
                            ln = min(8, nk - 8 * p)
                            eng2 = (
                                nc.gpsimd
                                if gps_only or (s + c + p) % 2 != 0
                                else nc.sync
                            )
                            eng2.dma_start(
                                phv[
                                    32 * s + p : 32 * s + p + 1,
                                    8 * c : 8 * c + ln,
                                    p * D : (p + 1) * D,
                                ],
                                tp16[o + 8 * p : o + 8 * p + ln, G, D : 2 * D],
                            )

                def emit_outers(T):
                    Bb, u = T // NSLOT, T % NSLOT
                    rows = tile_rows(T)
                    f_tiles[T] = fpsum.tile(
                        [128, FTILE * 128], F32, tag="f", name="ftile"
                    )
                    # group-boundary hooks (prefetch DMAs / projections)
                    for r in rows:
                        if r >= 0 and r % 128 == 0:
                            G = r // 128
                            if 5 <= G + 5 < ng:
                                emit_x1_dma(G + 5)
                            if 3 <= G + 3 < ng:
                                emit_proj_tp(G + 3)
                            if (G + 1) % gpq == 0 and (G + 1) // gpq < nq:
                                emit_x0_dma((G + 1) // gpq)
                            if G % gpq == 0 and G > 0:
                                emit_proj_g(G // gpq)
                            nd_tiles[G] = nd_all[
                                :, (G % 2) * 256 : (G % 2) * 256 + 256
                            ]
                            xr_tiles[G] = resid.tile([128, C], F16, tag="xr", name="xrt")
                            nc.sync.dma_start(
                                xr_tiles[G], x0r[G * 128 : (G + 1) * 128, :]
                            )
                    # prefetch realign chunks 2 ahead (chunk = 8 tiles)
                    if u % 8 == 0:
                        Tp = T + 16
                        if Tp < n_ftiles:
                            emit_realign(Tp // NSLOT, (Tp % NSLOT) // 8)
                    bi = Bb % 2
                    for s in range(NSTRIP):
                        if rows[4 * s] < 0:
                            break
                        nc.tensor.matmul(
                            f_tiles[T][:, s * 512 : (s + 1) * 512],
                            lhsT=thbuf[bi][32 * s : 32 * s + 32, u * D : (u + 1) * D],
                            rhs=phbuf[bi][
                                32 * s : 32 * s + 32, u * 4 * D : (u + 1) * 4 * D
                            ],
                            tile_position=(32 * s, 0),
                        )

                def emit_exp(T):
                    ncol = sum(128 for r in tile_rows(T) if r >= 0)
                    et = epool.tile([128, FTILE * 128], BF16, tag="e", name="etile")
                    if T % 5 in DVE_TILES:
                        nc.vector.tensor_scalar(
                            et.bitcast(I16)[:, :ncol],
                            f_tiles[T][:, :ncol],
                            SCH_A,
                            SCH_B,
                            mybir.AluOpType.mult,
                            mybir.AluOpType.add,
                        )
                    else:
                        nc.scalar.activation(
                            et[:, :ncol],
                            f_tiles[T][:, :ncol],
                            mybir.ActivationFunctionType.Exp,
                        )
                    e_tiles[T] = et

                def emit_reduces(T):
                    et = e_tiles[T]
                    for j, r in enumerate(tile_rows(T)):
                        if r < 0:
                            continue
                        G, bl = divmod(r, 128)
                        nc.tensor.matmul(
                            nd_tiles[G][:, 2 * bl : 2 * bl + 2],
                            lhsT=et[:, j * 128 : (j + 1) * 128],
                            rhs=g1[:, 2 * r : 2 * r + 2],
                        )

                def emit_final(G):
                    # borrow an f-pool PSUM slot for the single N=1024 bf16
                    # W-projection matmul, then add the residual on DVE
                    ot = osb.tile([128, C], F32, tag="ot", name="ott")
                    op = fpsum.tile([128, FTILE * 128], F32, tag="f", name="opt")
                    for h in range(2):
                        nc.tensor.matmul(
                            op[:, h * 512 : (h + 1) * 512],
                            lhsT=y16[:, G * 128 : (G + 1) * 128],
                            rhs=wwt_sb[:, h * 512 : (h + 1) * 512],
                        )
                    for h in range(2):
                        nc.vector.tensor_add(
                            ot[:, h * 512 : (h + 1) * 512],
                            op[:, h * 512 : (h + 1) * 512],
                            xr_tiles[G][:, h * 512 : (h + 1) * 512],
                        )
                    nc.sync.dma_start(out[G * 128 : (G + 1) * 128, :], ot)

                def emit_divide(G):
                    nd = ndsb.tile([128, 256], F32, tag="ndsb")
                    nc.scalar.copy(nd, nd_tiles[G])
                    ndv = nd.rearrange("p (b two) -> p b two", two=2)
                    r = rec.tile([128, 128], F32, tag="rec")
                    # den >= 1 always (sum of exps incl. the j=i term), so the
                    # fast approx (~51 ULP) has no edge cases here.
                    nc.vector.reciprocal_approx_fast(r, ndv[:, :, 1])
                    nc.vector.tensor_mul(
                        y16[:, G * 128 : (G + 1) * 128], ndv[:, :, 0], r
                    )

                def groups_done_at(Tr):
                    # groups whose reduces complete exactly at reduce-tile Tr
                    return [G for G, Td in g_done_tile.items() if Td == Tr]

                # ---- startup: input prefetch, projections, realign,
                # and PE warm-up matmuls to cover the DMA latency ----
                for Gp in range(min(3, ng)):
                    emit_x1_dma(Gp, nsplit=4)
                for Gp in range(3, min(5, ng)):
                    emit_x1_dma(Gp, nsplit=2)
                emit_x0_dma(0, nsplit=4)
                # warm-up: K=4 zero matmuls through all strips keep the PE
                # active (HAM un-throttles ~3.4us in) while DMAs land.
                nwarm = int(os.environ.get("K_WARM", "26"))
                if nwarm:
                    wpsum = fpsum.tile([128, FTILE * 128], F32, tag="f", name="warm")
                    for w in range(nwarm):
                        s = w % NSTRIP
                        nc.tensor.matmul(
                            wpsum[:, s * 512 : (s + 1) * 512],
                            lhsT=wbuf[32 * s : 32 * s + 32, 0:D],
                            rhs=wbuf[32 * s : 32 * s + 32, D : 5 * D],
                            tile_position=(32 * s, 0),
                        )
                # realign(0,0) only needs G0; emit it right after proj(G0)
                # so the PE isn't stuck behind a long cold projection burst.
                emit_proj_tp(0)
                emit_realign(0, 0, gps_only=True)
                for Gp in range(1, min(3, ng)):
                    emit_proj_tp(Gp)
                emit_realign(0, 1, gps_only=True)

                # software-pipelined emission: outers(T), exp(T-1),
                # reduces(T-LAG_RED); divide and final lag further behind so
                # the PE never waits on the exp output or DVE divide chain.
                # LAG_DIV=0: divide(G) must be emitted before the reduces of
                # G+2 (same nd-PSUM half) which start one tile later.
                LAG_RED, LAG_DIV, LAG_FIN = 4, 0, 6
                # NOTE: exp(T-1) is emitted BEFORE outers(T): with a 2-slot
                # f-pool, the slot outers(T) recycles must already have its
                # reader (the exp of the tile 2 allocations back) emitted.
                for T in range(n_ftiles + LAG_RED + LAG_FIN):
                    if 1 <= T <= n_ftiles:
                        emit_exp(T - 1)
                    for G in groups_done_at(T - LAG_RED - LAG_FIN):
                        emit_final(G)
                    if T < n_ftiles:
                        emit_outers(T)
                    if T == 1:
                        # deferred init work off the critical startup path:
                        # g1 ones (needed by reduces from T=4), phbuf[1]
                        # zeros (needed by realign of batch 1, ~T=16).
                        nc.vector.memset(g1, 1.0)
                        for s in range(NSTRIP):
                            nc.sync.dma_start(
                                phbuf[1][32 * s : 32 * s + 32, :], zz[:, :]
                            )
                            nc.sync.dma_start(
                                thbuf[1][32 * s + 4 : 32 * s + 32, :],
                                zz[0:28, : NSLOT * D],
                            )
                        emit_proj_g(0)  # x0 DMA was issued at startup; MMs here
                    if LAG_RED <= T < n_ftiles + LAG_RED:
                        emit_reduces(T - LAG_RED)
                    for G in groups_done_at(T - LAG_RED - LAG_DIV):
                        emit_divide(G)

    nc.compile()
    return nc


_BASS_CACHE = {}


def _get_bass(bc):
    if bc not in _BASS_CACHE:
        _BASS_CACHE[bc] = build_bass(bc)
    return _BASS_CACHE[bc]


def make_core_inputs(x0, x1, g_w, g_b, theta_w, theta_b, phi_w, phi_b, W_w, W_b,
                     bc=None, ncores=NCORES):
    """Host-side preprocessing -> list of per-core input dicts."""
    n = x0.shape[0] if bc is None else bc * ncores
    bc = n // ncores

    x0 = np.asarray(x0, np.float32)[:n]
    x1 = np.asarray(x1, np.float32)[:n]
    x1f = x1.astype(np.float16)
    x0f = x0.astype(np.float16)
    x0r = x0 if not np.any(W_b) else (x0 + np.asarray(W_b, np.float32)[None, :])
    x0r = np.ascontiguousarray(x0r, dtype=np.float16)

    KC = C // 128
    qsz = min(bc, 512)

    # per-partition-contiguous swizzles: arr[G*128+p, k*blk+b] = x[G*blk+b, k*128+p]
    def swizzle(xc, blk):
        g = xc.shape[0] // blk
        a = xc.reshape(g, blk, KC, 128)
        return np.ascontiguousarray(a.transpose(0, 3, 2, 1).reshape(g * 128, KC * blk))

    wc = np.concatenate(
        [np.asarray(theta_w).T, np.asarray(phi_w).T], axis=1
    ).astype(np.float16)  # [C, 2D]
    wcg = np.ascontiguousarray(
        wc.reshape(KC, 128, 2 * D).transpose(1, 0, 2).reshape(128, KC * 2 * D)
    )
    gwt = np.asarray(g_w).T.astype(np.float16)  # [C, D]
    gwtg = np.ascontiguousarray(
        gwt.reshape(KC, 128, D).transpose(1, 0, 2).reshape(128, KC * D)
    )
    import ml_dtypes
    wwt = np.ascontiguousarray(np.asarray(W_w).T.astype(ml_dtypes.bfloat16))  # [D, C]
    btp = np.ascontiguousarray(
        np.tile(np.concatenate([np.asarray(theta_b), np.asarray(phi_b)])[None, :], (128, 1)).astype(np.float32)
    )
    gbc = np.ascontiguousarray(np.asarray(g_b, np.float32).reshape(D, 1))
    zzc = np.zeros((32, NSLOT * 4 * D), np.float16)

    in_maps = []
    for c in range(ncores):
        sl = slice(c * bc, (c + 1) * bc)
        in_maps.append(
            {
                "x1g": swizzle(x1f[sl], 128),
                "x0g": swizzle(x0f[sl], qsz),
                "x0r": np.ascontiguousarray(x0r[sl]),
                "wc": wcg,
                "gwt": gwtg,
                "wwt": wwt,
                "btp": btp,
                "gb": gbc,
                "zz": zzc,
            }
        )
    return in_maps, bc


def kernel(x0, x1, g_w, g_b, theta_w, theta_b, phi_w, phi_b, W_w, W_b):
    from concourse.bass_utils import run_bass_kernel_spmd

    in_maps, bc = make_core_inputs(
        x0, x1, g_w, g_b, theta_w, theta_b, phi_w, phi_b, W_w, W_b
    )
    nc = _get_bass(bc)
    res = run_bass_kernel_spmd(nc, in_maps, core_ids=list(range(NCORES)))
    outs = [r["out"] for r in res.results]
    return np.ascontiguousarray(np.concatenate(outs, axis=0), dtype=np.float32)
